# revision 1
# baseline (speedup 1.0000x reference)
"""Causal GQA self-attention block (B=4, T=2048, C=1024, H=16, G=4) on 8
Trainium2 NeuronCores.

Sharding: core c = d*4+g  (d in {0,1} batch-DP, g in {0..3} kv-group TP).
Each core handles batches [2d, 2d+1], heads {g, g+4, g+8, g+12}, kv group g,
and produces a partial projection output; the host sums the 4 TP partials
per batch pair and adds the bias.

Per-core kernel (all matmuls in fp32r = 20-bit rounded fp32, exact PE
arithmetic on rounded inputs, fp32 PSUM accumulation):
  - fused QKV projection from pre-transposed x (host supplies x^T),
    producing Q^T / K^T / V^T with channels on partitions
  - scores computed transposed (S^T[tk,tq] = K Q^T) in 128x512 tiles,
    head-pair packed into the PE array via tile_position (contraction=64)
  - causal: block skip + column trim + additive -1e30 triangular band mask
  - unnormalized softmax: exp on ACT (scale folded), denominator obtained
    by appending a ones-column to V in the P@V matmul (M=65)
  - normalize via DVE reciprocal + gpsimd partition-broadcast + DVE mult
  - output projection on-device; host sums TP partials
"""

import os
import sys

sys.path.insert(0, "/opt/trn_rl_repo")

import numpy as np
from contextlib import ExitStack

import concourse.bass as bass
import concourse.mybir as mybir
import concourse.tile as tile
from concourse import bacc
from concourse.bass_utils import run_bass_kernel_spmd

# problem shape (hardcoded per contract)
B, T, C = 4, 2048, 1024
H, G = 16, 4
D = C // H  # 64

# per-core
B_LOC = 2        # batches per core
NPAIR = 2        # head pairs per core (4 heads)
P = 128
CC = C // P      # 8 contraction chunks for projections
NT = 512         # tq tile width
TQT = T // NT    # 4 tq tiles
TKC = T // P     # 16 tk chunks
NEG = -1.0e30

F32 = mybir.dt.float32
F32R = mybir.dt.float32r
BF16 = mybir.dt.bfloat16
# bf16 for the attention operands (Q/K/V/P): enables fast weight loads on the
# PE; projections stay fp32r. End-to-end absmax-rel error ~2e-3 (vs 2e-4).
ATTN_BF16 = True
ADT = BF16 if ATTN_BF16 else F32R
Exp = mybir.ActivationFunctionType.Exp
ADD = mybir.AluOpType.add
MULT = mybir.AluOpType.mult


def round_fp32r(a: np.ndarray) -> np.ndarray:
    """Round fp32 to fp32r (1s+8e+11m kept in the high 20 bits), RTNE.
    Matches the hardware DVE fp32->fp32r cast bit-exactly (probed)."""
    u = np.ascontiguousarray(a, dtype=np.float32).view(np.uint32)
    tie = (u >> np.uint32(12)) & np.uint32(1)
    r = (u + np.uint32(0x7FF) + tie) & np.uint32(0xFFFFF000)
    return r.view(np.float32)


def _build_program():
    nc = bacc.Bacc(None, target_bir_lowering=False)

    xT = nc.dram_tensor("xT", [B_LOC, C, T], ADT, kind="ExternalInput")
    # columns: q pair0 (128) | q pair1 (128) | k (64) | v (64)
    wqkv = nc.dram_tensor("wqkv", [C, 384], ADT, kind="ExternalInput")
    wproj = nc.dram_tensor("wproj", [2 * P, C], ADT, kind="ExternalInput")
    # multiplicative triangular band mask, duplicated for the 2 packed heads
    maskb = nc.dram_tensor("maskb", [P, 2, P], ADT, kind="ExternalInput")
    ident2 = nc.dram_tensor("ident2", [P, 64], ADT, kind="ExternalInput")
    vones = nc.dram_tensor("vones", [P, TKC], ADT, kind="ExternalInput")
    outp = nc.dram_tensor("outp", [B_LOC, T, C], F32, kind="ExternalOutput")

    with tile.TileContext(nc) as tc:
        with ExitStack() as ctx:
            const = ctx.enter_context(tc.tile_pool(name="const", bufs=1))
            sb = ctx.enter_context(tc.tile_pool(name="sb", bufs=1))
            sb2 = ctx.enter_context(tc.tile_pool(name="sb2", bufs=2))
            xp = ctx.enter_context(tc.tile_pool(name="xp", bufs=2))
            small = ctx.enter_context(tc.tile_pool(name="small", bufs=4))
            ppool = ctx.enter_context(tc.tile_pool(name="ppool", bufs=4))
            stg = ctx.enter_context(tc.tile_pool(name="stg", bufs=3))
            ps_st = ctx.enter_context(tc.tile_pool(name="ps_st", bufs=2, space="PSUM"))
            ps_pv = ctx.enter_context(tc.tile_pool(name="ps_pv", bufs=2, space="PSUM"))
            ps_mm = ctx.enter_context(tc.tile_pool(name="ps_mm", bufs=2, space="PSUM"))

            # ---- constants ----
            wqkv_t = const.tile([P, CC, 384], ADT, tag="wqkv")
            for cc in range(CC):
                nc.sync.dma_start(wqkv_t[:, cc, :], wqkv[cc * P : (cc + 1) * P, :])
            wproj_t = const.tile([P, 2, C], ADT, tag="wproj")
            for cc in range(2):
                nc.sync.dma_start(wproj_t[:, cc, :], wproj[cc * P : (cc + 1) * P, :])
            mask_t = const.tile([P, 2, P], ADT, tag="maskb")
            nc.sync.dma_start(mask_t[:], maskb[:])
            id2_t = const.tile([P, 64], ADT, tag="ident2")
            nc.sync.dma_start(id2_t[:], ident2[:])

            def emit_setup(b):
                # ---- load x^T for this batch + allocate state ----
                xt = xp.tile([P, CC, T], ADT, tag="xt")
                for cc in range(CC):
                    eng = nc.gpsimd if cc % 2 else nc.sync
                    eng.dma_start(xt[:, cc, :], xT[b, cc * P : (cc + 1) * P, :])
                # q_sb[:, p, t]: pair p -> heads (2p, 2p+1) at rows 0:64 / 64:128
                q_sb = sb2.tile([P, NPAIR, T], ADT, tag="q")
                # kv_sb rows 0:64 = K^T (kv-group), rows 64:128 = V^T
                kv_sb = sb2.tile([P, TQT, NT], ADT, tag="kv")
                k_hi = sb2.tile([P, TQT, NT], ADT, tag="khi")  # K dup at rows 64:128
                v_a = sb2.tile([P, TKC, 65], ADT, tag="va")
                nc.sync.dma_start(v_a[:, :, 64], vones[:])
                o_t = sb2.tile([P, NPAIR, T], ADT, tag="ot", name=f"ot{b}")
                return xt, q_sb, kv_sb, k_hi, v_a, o_t

            def emit_qkv_part(b, st8, n, part):
                # ---- QKV projection tile n, sub-part (0: kv proj + V
                # transpose + k dup, 1: q pair0 proj, 2: q pair1 proj) ----
                xt, q_sb, kv_sb, k_hi, v_a, o_t = st8
                m = {0: 2, 1: 0, 2: 1}[part]
                pm = ps_mm.tile([P, NT], F32, tag="mm")
                for cc in range(CC):
                    nc.tensor.matmul(
                        pm[:],
                        wqkv_t[:, cc, m * P : (m + 1) * P],
                        xt[:, cc, n * NT : (n + 1) * NT],
                        start=(cc == 0),
                        stop=(cc == CC - 1),
                    )
                if m < 2:
                    nc.vector.tensor_copy(q_sb[:, m, n * NT : (n + 1) * NT], pm[:])
                    return
                nc.vector.tensor_copy(kv_sb[:, n, :], pm[:])
                nc.sync.dma_start(k_hi[64:128, n, :], kv_sb[0:64, n, :])
                for i in range(4 * n, 4 * n + 4):
                    pt = ps_mm.tile([P, 64], ADT, tag="mm")
                    nc.tensor.transpose(
                        pt[:],
                        kv_sb[64:128, i // 4, (i % 4) * P : (i % 4 + 1) * P],
                        id2_t[64:128, :],
                    )
                    nc.vector.tensor_copy(v_a[:, i, 0:64], pt[:])

            def emit_attn_jp(b, st8, j, p_):
                xt, q_sb, kv_sb, k_hi, v_a, o_t = st8
                if True:
                    if True:
                        pv = [
                            ps_pv.tile([P, NT], F32, tag="pv", name=f"pv{e}")
                            for e in range(2)
                        ]
                        last = 4 * j + 3
                        for i in range(4 * j + 4):
                            diag = i >= 4 * j
                            r = i - 4 * j
                            lo = r * P if diag else 0
                            st = ps_st.tile([P, 2, NT], F32, tag="st")
                            for e in range(2):
                                ksrc = kv_sb if e == 0 else k_hi
                                nc.tensor.matmul(
                                    st[:, e, lo:NT],
                                    ksrc[
                                        64 * e : 64 * e + 64,
                                        i // 4,
                                        (i % 4) * P : (i % 4 + 1) * P,
                                    ],
                                    q_sb[
                                        64 * e : 64 * e + 64,
                                        p_,
                                        j * NT + lo : (j + 1) * NT,
                                    ],
                                    start=True,
                                    stop=True,
                                    tile_position=(64 * e, 0),
                                )
                            pexp = ppool.tile([P, 2, NT], ADT, tag="pexp")
                            nc.scalar.activation(
                                pexp[:, :, lo:NT],
                                st[:, :, lo:NT],
                                Exp,
                                scale=0.125,
                            )
                            if diag:
                                nc.vector.tensor_tensor(
                                    pexp[:, :, lo : lo + P],
                                    pexp[:, :, lo : lo + P],
                                    mask_t[:],
                                    MULT,
                                )
                            for e in range(2):
                                nc.tensor.matmul(
                                    pv[e][0:65, lo:NT],
                                    v_a[:, i, :],
                                    pexp[:, e, lo:NT],
                                    start=(i == 0),
                                    stop=(i == last),
                                )
                        # normalize: o = pv[0:64] / pv[64]
                        for e in range(2):
                            # copy psum out early to release the PV bank
                            pvs = small.tile([65, NT], F32, tag="pvs", name=f"pvs{e}")
                            nc.vector.tensor_copy(pvs[:], pv[e][0:65, :])
                            # reciprocal_approx_fast and partition_broadcast
                            # both require absolute partition 0 on HW: shift
                            # the denominator row down first
                            l0 = small.tile([1, NT], F32, tag="l0")
                            nc.sync.dma_start(l0[:], pvs[64:65, :])
                            rec0 = small.tile([1, NT], F32, tag="rec0")
                            nc.vector.reciprocal_approx_fast(rec0[:], l0[:])
                            bca = small.tile([64, NT], F32, tag="bca")
                            nc.gpsimd.partition_broadcast(bca[:], rec0[:])
                            if e == 0:
                                nc.vector.tensor_tensor(
                                    o_t[0:64, p_, j * NT : (j + 1) * NT],
                                    pvs[0:64, :],
                                    bca[:],
                                    MULT,
                                )
                            else:
                                otmp = small.tile([64, NT], ADT, tag="otmp")
                                nc.vector.tensor_tensor(
                                    otmp[:], pvs[0:64, :], bca[:], MULT
                                )
                                nc.sync.dma_start(
                                    o_t[64:128, p_, j * NT : (j + 1) * NT], otmp[:]
                                )

            def emit_proj_t(b, st8, t_):
                # ---- output projection for one tq chunk (partial) ----
                o_t = st8[5]
                if True:
                    stage = stg.tile([P, C], F32, tag="stage")
                    for n2 in range(2):
                        pm = ps_mm.tile([P, NT], F32, tag="mm")
                        for cc2 in range(2):
                            nc.tensor.matmul(
                                pm[:],
                                o_t[:, cc2, t_ * P : (t_ + 1) * P],
                                wproj_t[:, cc2, n2 * NT : (n2 + 1) * NT],
                                start=(cc2 == 0),
                                stop=(cc2 == 1),
                            )
                        nc.vector.tensor_copy(stage[:, n2 * NT : (n2 + 1) * NT], pm[:])
                    nc.sync.dma_start(outp[b, t_ * P : (t_ + 1) * P, :], stage[:])

            # fully phase-pipelined emission: QKV parts and proj chunks are
            # threaded between attention (j, pair) blocks so the ACT (exp)
            # stream never waits on a solid PE-only phase
            def schedule(b, st8, nxt):
                # hand-tuned fill plan: qkv(n) lands before attn tile n, proj
                # chunks lag their attention tile by ~1 tile of drain time;
                # the next batch's setup + first qkv ride inside tiles 2-3
                QK = lambda n, p: ("qkv", n, p)
                PR = lambda t: ("proj", t)
                SU = ("setup",)
                NX = lambda p: ("qkvn", p)
                plan = {
                    (0, 0): [QK(1, 0)],
                    (0, 1): [QK(1, 1), QK(1, 2)],
                    (1, 0): [QK(2, 0), QK(2, 1)],
                    (1, 1): [QK(2, 2), QK(3, 0), PR(0), PR(1)],
                    (2, 0): [QK(3, 1), QK(3, 2), PR(2), PR(3)],
                    (2, 1): [SU, NX(0), PR(4), PR(5)],
                    (3, 0): [NX(1), NX(2), PR(6), PR(7)],
                    (3, 1): [PR(8), PR(9), PR(10), PR(11)],
                }
                st_n = None
                for j in range(TQT):
                    for p_ in range(NPAIR):
                        emit_attn_jp(b, st8, j, p_)
                        for f in plan[(j, p_)]:
                            if f[0] == "qkv":
                                emit_qkv_part(b, st8, f[1], f[2])
                            elif f[0] == "proj":
                                emit_proj_t(b, st8, f[1])
                            elif f[0] == "setup" and nxt is not None:
                                st_n = emit_setup(nxt)
                            elif f[0] == "qkvn" and nxt is not None:
                                emit_qkv_part(nxt, st_n, 0, f[1])
                for t_ in range(12, 16):
                    emit_proj_t(b, st8, t_)
                return st_n

            st0 = emit_setup(0)
            emit_qkv_part(0, st0, 0, 0)
            emit_qkv_part(0, st0, 0, 1)
            emit_qkv_part(0, st0, 0, 2)
            st1 = schedule(0, st0, 1)
            schedule(1, st1, None)

    nc.compile()
    return nc


_NC = None


def _get_program():
    global _NC
    if _NC is None:
        _NC = _build_program()
    return _NC


def _host_inputs(x, Wq, Wkv, Wproj):
    """Shard + lay out inputs for the 8 cores."""
    # triangular band mask (additive): 0 where tk_l <= tq_l else -1e30,
    # duplicated for the two packed heads
    import ml_dtypes

    adt_np = ml_dtypes.bfloat16 if ATTN_BF16 else np.float32
    tri = np.where(
        np.arange(P)[:, None] <= np.arange(P)[None, :], 1.0, 0.0
    ).astype(np.float32)
    ident2 = np.concatenate([np.eye(64, dtype=np.float32)] * 2, axis=0).astype(
        adt_np
    )  # [128, 64]
    maskb = np.stack([tri, tri], axis=1).astype(adt_np)  # [128, 2, 128]

    in_maps = []
    for d in range(2):
        xT = x[2 * d : 2 * d + 2].transpose(0, 2, 1).astype(adt_np)
        for g in range(G):
            heads = [g, g + 4, g + 8, g + 12]
            wq_cols = np.concatenate(
                [Wq[h * D : (h + 1) * D, :] for h in heads], axis=0
            ).T  # [1024, 256]
            wk = Wkv[g * D : (g + 1) * D, :].T  # [1024, 64]
            wv = Wkv[G * D + g * D : G * D + (g + 1) * D, :].T
            wqkv = np.concatenate([wq_cols, wk, wv], axis=1).astype(adt_np)
            ch = np.concatenate(
                [np.arange(h * D, (h + 1) * D) for h in heads]
            )
            wproj_s = np.ascontiguousarray(Wproj[:, ch].T).astype(adt_np)
            in_maps.append(
                {
                    "xT": xT,
                    "wqkv": wqkv,
                    "wproj": wproj_s,
                    "maskb": maskb,
                    "ident2": ident2,
                    "vones": np.ones((P, TKC), dtype=adt_np),
                }
            )
    return in_maps


def kernel(x, Wq, Wkv, Wproj, b_proj):
    x = np.asarray(x, dtype=np.float32)
    Wq = np.asarray(Wq, dtype=np.float32)
    Wkv = np.asarray(Wkv, dtype=np.float32)
    Wproj = np.asarray(Wproj, dtype=np.float32)
    b_proj = np.asarray(b_proj, dtype=np.float32)

    nc = _get_program()
    in_maps = _host_inputs(x, Wq, Wkv, Wproj)
    trace = bool(int(os.environ.get("BASS_KERNEL_TRACE", "0")))
    res = run_bass_kernel_spmd(nc, in_maps, list(range(8)), trace=trace)
    if trace:
        kernel.last_results = res

    out = np.empty((B, T, C), dtype=np.float32)
    for d in range(2):
        acc = res.results[4 * d]["outp"].astype(np.float32).copy()
        for g in range(1, G):
            acc += res.results[4 * d + g]["outp"]
        out[2 * d : 2 * d + 2] = acc + b_proj[None, None, :]
    return out

